# revision 1
# baseline (speedup 1.0000x reference)
"""Trainium2 Bass kernel for GTStepwiseConstantVelocityModel.

Strategy: shard node-pair work across 8 cores via a circulant pairing
(node n owns pairs (n, (n+d) mod 384) for d=1..191; pairs at d=192 are a
separate strip split across cores). Each core gets row-rotated copies of
the inputs so the compiled SPMD program is identical on every core.
Layout on device: t (=128) on partitions, node columns on the free axis.
Step positions come from a lower-triangular matmul on the TensorEngine
(cumsum), the elementwise intensity-integral pipeline runs on DVE+ACT
(ln/exp instead of sqrt/rsqrt to stay in one ACT table set), and the
event term is computed with host-built one-hot gather matmuls. Each core
emits [event_partial, nonevent_partial]; the host sums the 8 pairs.
"""
import numpy as np

N, D, T, E, NC = 384, 2, 128, 256, 8
EXTW = 576          # extended (wrapped) column count
G = 4               # rows per chunk in the main loop
NROW = N // NC      # 48 rows per core
EV_PER = E // NC    # 32 events per core
SP_PER = 192 // NC  # 24 strip pairs per core
F = G * 191

_CACHE = {}


def _build_program(dt):
    from contextlib import ExitStack
    import concourse.bacc as bacc
    import concourse.tile as tile
    import concourse.mybir as mybir

    f32 = mybir.dt.float32
    AF = mybir.ActivationFunctionType
    OP = mybir.AluOpType
    AX = mybir.AxisListType
    LN_SPI2 = float(np.log(np.sqrt(np.pi) / 2.0))

    nc = bacc.Bacc("TRN2", target_bir_lowering=False, debug=False, num_devices=NC)

    def din(name, shape):
        return nc.dram_tensor(name, shape, f32, kind="ExternalInput").ap()

    vxe_d = din("vxe", [T, EXTW])
    vye_d = din("vye", [T, EXTW])
    z0xe_d = din("z0xe", [1, EXTW])
    z0ye_d = din("z0ye", [1, EXTW])
    lmat_d = din("lmat", [T, T])
    vstrip_d = din("vstrip", [T, 4 * SP_PER])
    z0strip_d = din("z0strip", [1, 4 * SP_PER])
    vnatx_d = din("vnatx", [N, T])
    vnaty_d = din("vnaty", [N, T])
    qmat_d = din("qmat", [N, EV_PER])
    wmat_d = din("wmat", [T, EV_PER])
    bhot_d = din("bhot", [T, EV_PER])
    dz0x_d = din("dz0x", [1, EV_PER])
    dz0y_d = din("dz0y", [1, EV_PER])
    betac_d = din("betac", [T, 1])
    out_d = nc.dram_tensor("out", [1, 2], f32, kind="ExternalOutput").ap()

    with ExitStack() as ctx:
        tc = ctx.enter_context(tile.TileContext(nc))
        sg = ctx.enter_context(tc.tile_pool(name="singles", bufs=1))
        wk = ctx.enter_context(tc.tile_pool(name="work", bufs=2))
        ps = ctx.enter_context(tc.tile_pool(name="psum", bufs=1, space="PSUM"))

        def load(dram, shape, tag):
            t = sg.tile(shape, f32, tag=tag)
            nc.sync.dma_start(out=t[:], in_=dram[:])
            return t

        vxe = load(vxe_d, [T, EXTW], "vxe")
        vye = load(vye_d, [T, EXTW], "vye")
        z0xe = load(z0xe_d, [1, EXTW], "z0xe")
        z0ye = load(z0ye_d, [1, EXTW], "z0ye")
        lmat = load(lmat_d, [T, T], "lmat")
        vstrip = load(vstrip_d, [T, 4 * SP_PER], "vstrip")
        z0strip = load(z0strip_d, [1, 4 * SP_PER], "z0strip")
        vnx = [sg.tile([128, T], f32, name=f"vnx{r}", tag=f"vnx{r}") for r in range(3)]
        vny = [sg.tile([128, T], f32, name=f"vny{r}", tag=f"vny{r}") for r in range(3)]
        qm = [sg.tile([128, EV_PER], f32, name=f"qm{r}", tag=f"qm{r}") for r in range(3)]
        for r in range(3):
            nc.sync.dma_start(out=vnx[r][:], in_=vnatx_d[128 * r:128 * (r + 1), :])
            nc.sync.dma_start(out=vny[r][:], in_=vnaty_d[128 * r:128 * (r + 1), :])
            nc.sync.dma_start(out=qm[r][:], in_=qmat_d[128 * r:128 * (r + 1), :])
        wmat = load(wmat_d, [T, EV_PER], "wmat")
        bhot = load(bhot_d, [T, EV_PER], "bhot")
        dz0x = load(dz0x_d, [1, EV_PER], "dz0x")
        dz0y = load(dz0y_d, [1, EV_PER], "dz0y")
        betac = load(betac_d, [T, 1], "betac")

        ones = sg.tile([T, 1], f32)
        nc.vector.memset(ones[:], 1.0)
        lones = sg.tile([1, T], f32)
        nc.vector.memset(lones[:], 1.0)
        zcol = sg.tile([T, 1], f32)
        nc.vector.memset(zcol[:], 0.0)
        epscol = sg.tile([T, 1], f32)
        nc.vector.memset(epscol[:], 1e-12)
        bln = sg.tile([T, 1], f32)
        nc.vector.tensor_scalar_add(out=bln[:], in0=betac[:], scalar1=LN_SPI2)

        # ---- step positions ZxE/ZyE via triangular matmul (cumsum + z0) ----
        zxe = sg.tile([T, EXTW], f32)
        zye = sg.tile([T, EXTW], f32)
        for (vsrc, zrow, zdst) in ((vxe, z0xe, zxe), (vye, z0ye, zye)):
            for fc in range(2):
                cs = slice(288 * fc, 288 * (fc + 1))
                pz = ps.tile([T, 288], f32)
                nc.tensor.matmul(pz[:], lmat[0:127, :], vsrc[0:127, cs],
                                 start=True, stop=False)
                nc.tensor.matmul(pz[:], lones[:], zrow[:, cs],
                                 start=False, stop=True)
                nc.scalar.copy(zdst[:, cs], pz[:])

        # strip step positions
        pzs = ps.tile([T, 4 * SP_PER], f32)
        nc.tensor.matmul(pzs[:], lmat[0:127, :], vstrip[0:127, :],
                         start=True, stop=False)
        nc.tensor.matmul(pzs[:], lones[:], z0strip[:],
                         start=False, stop=True)
        zstrip = sg.tile([T, 4 * SP_PER], f32)
        nc.scalar.copy(zstrip[:], pzs[:])

        # ---- event term ----
        gdx = ps.tile([T, EV_PER], f32)
        gdy = ps.tile([T, EV_PER], f32)
        for r in range(3):
            nc.tensor.matmul(gdx[:], vnx[r][:], qm[r][:],
                             start=(r == 0), stop=(r == 2))
        for r in range(3):
            nc.tensor.matmul(gdy[:], vny[r][:], qm[r][:],
                             start=(r == 0), stop=(r == 2))
        hx = wk.tile([T, EV_PER], f32, tag="hx")
        hy = wk.tile([T, EV_PER], f32, tag="hy")
        nc.vector.tensor_mul(hx[:], gdx[:], wmat[:])
        nc.vector.tensor_mul(hy[:], gdy[:], wmat[:])
        shx = ps.tile([1, EV_PER], f32)
        shy = ps.tile([1, EV_PER], f32)
        brow = ps.tile([1, EV_PER], f32)
        nc.tensor.matmul(shx[:], ones[:], hx[:])
        nc.tensor.matmul(shy[:], ones[:], hy[:])
        nc.tensor.matmul(brow[:], betac[:], bhot[:])
        evx = sg.tile([1, EV_PER], f32)
        evy = sg.tile([1, EV_PER], f32)
        nc.vector.tensor_add(evx[:], shx[:], dz0x[:])
        nc.vector.tensor_add(evy[:], shy[:], dz0y[:])
        nc.vector.tensor_mul(evx[:], evx[:], evx[:])
        nc.vector.tensor_mul(evy[:], evy[:], evy[:])
        nc.vector.tensor_add(evx[:], evx[:], evy[:])
        evel = sg.tile([1, EV_PER], f32)
        nc.vector.tensor_sub(evel[:], brow[:], evx[:])
        ev_s = sg.tile([1, 1], f32)
        nc.vector.reduce_sum(out=ev_s[:], in_=evel[:], axis=AX.X)

        # ---- main circulant pipeline ----
        part_cols = []

        def pipeline(fw, dvx, dvy, dzx, dzy):
            s1 = wk.tile([T, F], f32, tag="s1")
            s2 = wk.tile([T, F], f32, tag="s2")
            nc.scalar.activation(s1[:, :fw], dvx, AF.Square, bias=zcol[:])
            nc.scalar.activation(s2[:, :fw], dvy, AF.Square, bias=zcol[:])
            a2 = wk.tile([T, F], f32, tag="a2")
            nc.vector.tensor_add(a2[:, :fw], s1[:, :fw], s2[:, :fw])
            lg = wk.tile([T, F], f32, tag="lg")
            nc.scalar.activation(lg[:, :fw], a2[:, :fw], AF.Ln, bias=epscol[:])
            av = wk.tile([T, F], f32, tag="av")
            nc.scalar.activation(av[:, :fw], lg[:, :fw], AF.Exp, scale=0.5, bias=zcol[:])
            inva = wk.tile([T, F], f32, tag="inva")
            nc.scalar.activation(inva[:, :fw], lg[:, :fw], AF.Exp, scale=-0.5, bias=zcol[:])
            bp = wk.tile([T, F], f32, tag="bp")
            bq = wk.tile([T, F], f32, tag="bq")
            nc.vector.tensor_mul(bp[:, :fw], dzx, dvx)
            nc.vector.tensor_mul(bq[:, :fw], dzy, dvy)
            bv = wk.tile([T, F], f32, tag="bv")
            nc.vector.tensor_add(bv[:, :fw], bp[:, :fw], bq[:, :fw])
            arg2 = wk.tile([T, F], f32, tag="arg2")
            nc.vector.tensor_mul(arg2[:, :fw], bv[:, :fw], inva[:, :fw])
            # r2 (reuse s1/s2/bp)
            nc.scalar.activation(s1[:, :fw], dzx, AF.Square, bias=zcol[:])
            nc.scalar.activation(s2[:, :fw], dzy, AF.Square, bias=zcol[:])
            r2 = wk.tile([T, F], f32, tag="r2")
            nc.vector.tensor_add(r2[:, :fw], s1[:, :fw], s2[:, :fw])
            sqa2 = wk.tile([T, F], f32, tag="sqa2")
            nc.scalar.activation(sqa2[:, :fw], arg2[:, :fw], AF.Square, bias=zcol[:])
            mres = wk.tile([T, F], f32, tag="mres")
            nc.vector.tensor_sub(mres[:, :fw], r2[:, :fw], sqa2[:, :fw])
            arg1 = wk.tile([T, F], f32, tag="arg1")
            nc.vector.scalar_tensor_tensor(
                out=arg1[:, :fw], in0=av[:, :fw], scalar=float(dt),
                in1=arg2[:, :fw], op0=OP.mult, op1=OP.add)
            wv = wk.tile([T, F], f32, tag="wv")
            nc.vector.scalar_tensor_tensor(
                out=wv[:, :fw], in0=lg[:, :fw], scalar=0.5,
                in1=mres[:, :fw], op0=OP.mult, op1=OP.add)
            exiv = wk.tile([T, F], f32, tag="exiv")
            nc.scalar.activation(exiv[:, :fw], wv[:, :fw], AF.Exp,
                                 bias=bln[:], scale=-1.0)
            e1 = wk.tile([T, F], f32, tag="e1")
            e2 = wk.tile([T, F], f32, tag="e2")
            nc.scalar.activation(e1[:, :fw], arg1[:, :fw], AF.Erf, bias=zcol[:])
            nc.scalar.activation(e2[:, :fw], arg2[:, :fw], AF.Erf, bias=zcol[:])
            ed = wk.tile([T, F], f32, tag="ed")
            nc.vector.tensor_sub(ed[:, :fw], e1[:, :fw], e2[:, :fw])
            t4 = wk.tile([T, F], f32, tag="t4")
            nc.vector.tensor_mul(t4[:, :fw], ed[:, :fw], exiv[:, :fw])
            col = sg.tile([T, 1], f32, name=f"col{len(part_cols)}", tag=f"col{len(part_cols)}")
            nc.vector.reduce_sum(out=col[:], in_=t4[:, :fw], axis=AX.X)
            part_cols.append(col)

        for ci in range(NROW // G):
            dvx = wk.tile([T, F], f32, tag="dvx")
            dvy = wk.tile([T, F], f32, tag="dvy")
            dzx = wk.tile([T, F], f32, tag="dzx")
            dzy = wk.tile([T, F], f32, tag="dzy")
            for g in range(G):
                k = ci * G + g
                j0 = 8 * k
                s = slice(191 * g, 191 * (g + 1))
                cs = slice(j0 + 1, j0 + 192)
                nc.vector.tensor_scalar_sub(out=dvx[:, s], in0=vxe[:, cs],
                                            scalar1=vxe[:, j0:j0 + 1])
                nc.vector.tensor_scalar_sub(out=dvy[:, s], in0=vye[:, cs],
                                            scalar1=vye[:, j0:j0 + 1])
                nc.vector.tensor_scalar_sub(out=dzx[:, s], in0=zxe[:, cs],
                                            scalar1=zxe[:, j0:j0 + 1])
                nc.vector.tensor_scalar_sub(out=dzy[:, s], in0=zye[:, cs],
                                            scalar1=zye[:, j0:j0 + 1])
            pipeline(F, dvx[:], dvy[:], dzx[:], dzy[:])

        # strip pipeline (width 24)
        sw = SP_PER
        dvxs = wk.tile([T, sw], f32, tag="dvxs")
        dvys = wk.tile([T, sw], f32, tag="dvys")
        dzxs = wk.tile([T, sw], f32, tag="dzxs")
        dzys = wk.tile([T, sw], f32, tag="dzys")
        nc.vector.tensor_sub(dvxs[:], vstrip[:, 0:sw], vstrip[:, sw:2 * sw])
        nc.vector.tensor_sub(dvys[:], vstrip[:, 2 * sw:3 * sw], vstrip[:, 3 * sw:4 * sw])
        nc.vector.tensor_sub(dzxs[:], zstrip[:, 0:sw], zstrip[:, sw:2 * sw])
        nc.vector.tensor_sub(dzys[:], zstrip[:, 2 * sw:3 * sw], zstrip[:, 3 * sw:4 * sw])
        pipeline(sw, dvxs[:], dvys[:], dzxs[:], dzys[:])

        # ---- reduce partials and write out ----
        while len(part_cols) > 1:
            nxt = []
            for i in range(0, len(part_cols) - 1, 2):
                dst = sg.tile([T, 1], f32, name=f"red{len(nxt)}_{len(part_cols)}", tag=f"red{len(nxt)}_{len(part_cols)}")
                nc.vector.tensor_add(dst[:], part_cols[i][:], part_cols[i + 1][:])
                nxt.append(dst)
            if len(part_cols) % 2:
                nxt.append(part_cols[-1])
            part_cols = nxt
        s_ps = ps.tile([1, 1], f32)
        nc.tensor.matmul(s_ps[:], part_cols[0][:], ones[:])
        out_sb = sg.tile([1, 2], f32)
        nc.vector.tensor_copy(out_sb[:, 0:1], ev_s[:])
        nc.vector.tensor_copy(out_sb[:, 1:2], s_ps[:])
        nc.sync.dma_start(out=out_d[:], in_=out_sb[:])

    nc.finalize()
    return nc


def _host_prep(data, t0, tn, z0, v0, beta):
    dt = float(tn - t0) / T
    v0x, v0y = np.ascontiguousarray(v0[:, 0, :]), np.ascontiguousarray(v0[:, 1, :])
    z0x, z0y = z0[:, 0], z0[:, 1]

    lmat = np.zeros((T, T), np.float32)
    for k in range(T - 1):
        lmat[k, k + 1:] = dt
    lmat[T - 1, :] = 1.0

    times = data[:, 2]
    idx_f = np.floor(times / dt)
    idx = np.where(idx_f < T, idx_f, idx_f - 1.0).astype(np.int32)
    rem = (times - idx_f * dt).astype(np.float32)
    i_idx = np.floor(data[:, 0]).astype(np.int32)
    j_idx = np.floor(data[:, 1]).astype(np.int32)

    in_maps = []
    for c in range(NC):
        ridx = (np.arange(EXTW) + c) % N
        m = {
            "vxe": np.ascontiguousarray(v0x[ridx, :].T),
            "vye": np.ascontiguousarray(v0y[ridx, :].T),
            "z0xe": np.ascontiguousarray(z0x[ridx][None, :]),
            "z0ye": np.ascontiguousarray(z0y[ridx][None, :]),
            "lmat": lmat,
            "vnatx": v0x, "vnaty": v0y,
            "betac": np.ascontiguousarray(beta[:, None]),
        }
        sA = np.arange(SP_PER * c, SP_PER * (c + 1))
        sB = sA + 192
        m["vstrip"] = np.ascontiguousarray(
            np.concatenate([v0x[sA].T, v0x[sB].T, v0y[sA].T, v0y[sB].T], axis=1))
        m["z0strip"] = np.concatenate(
            [z0x[sA], z0x[sB], z0y[sA], z0y[sB]])[None, :].astype(np.float32)
        es = slice(EV_PER * c, EV_PER * (c + 1))
        ii, jj, dd, rr = i_idx[es], j_idx[es], idx[es], rem[es]
        Q = np.zeros((N, EV_PER), np.float32)
        W = np.zeros((T, EV_PER), np.float32)
        B = np.zeros((T, EV_PER), np.float32)
        for e in range(EV_PER):
            Q[ii[e], e] += 1.0
            Q[jj[e], e] -= 1.0
            W[:dd[e], e] = dt
            W[dd[e], e] += rr[e]
            B[dd[e], e] = 1.0
        m["qmat"], m["wmat"], m["bhot"] = Q, W, B
        m["dz0x"] = (z0x[ii] - z0x[jj])[None, :].astype(np.float32)
        m["dz0y"] = (z0y[ii] - z0y[jj])[None, :].astype(np.float32)
        in_maps.append({k: np.ascontiguousarray(v, dtype=np.float32)
                        for k, v in m.items()})
    return dt, in_maps


def _run(inputs, trace=False):
    from concourse.bass_utils import run_bass_kernel_spmd
    data = np.asarray(inputs["data"], np.float32)
    t0 = float(np.asarray(inputs["t0"]))
    tn = float(np.asarray(inputs["tn"]))
    z0 = np.asarray(inputs["z0"], np.float32)
    v0 = np.asarray(inputs["v0"], np.float32)
    beta = np.asarray(inputs["beta"], np.float32)

    dt, in_maps = _host_prep(data, t0, tn, z0, v0, beta)
    if dt not in _CACHE:
        _CACHE[dt] = _build_program(dt)
    nc = _CACHE[dt]
    res = run_bass_kernel_spmd(nc, in_maps, core_ids=list(range(NC)), trace=trace)
    ev = sum(float(res.results[c]["out"][0, 0]) for c in range(NC))
    S = sum(float(res.results[c]["out"][0, 1]) for c in range(NC))
    return np.array(np.float32(ev - S)), res


def kernel(**inputs):
    out, _ = _run(inputs, trace=False)
    return out



# revision 24
# speedup vs baseline: 1.5566x; 1.5566x over previous
"""Trainium2 Bass kernel for GTStepwiseConstantVelocityModel.

Strategy: circulant pairing — node n owns pairs (n, (n+d) mod 384) for
d=1..191; the d=192 strip is split 24 pairs/core. 8 cores each take a
48-row block. Layout: t (=128) on partitions, pair columns on the free
axis, bf16 throughout the elementwise pipeline (final sums in fp32).

Per pair-step the closed-form integral needs a2=|dv|^2, r2=|dz|^2,
b=dz.dv, then erf/exp evaluation. a2/r2 come from a fused custom DVE op
(squared-diff with per-partition column bias + eps floor); b comes from
two fused diff-product customs (direct products — polarization in bf16
would catastrophically cancel). ACT does only AbsRsqrt/Erf/Square/Exp
with table-phased emission (3 table loads per 3-chunk group); Pool
(GpSimd) absorbs some elementwise passes; the final multiply+reduce is
a fused tensor_tensor_reduce. Events via host-built one-hot matmuls on
the PE (as in the original baseline). Host sums 8 per-core partials.
"""
import os
import numpy as np

KNOCUSTOM = os.environ.get("KNOCUSTOM", "0") == "1"
KNOTTR = os.environ.get("KNOTTR", "1") == "1"
KNOPOOL = os.environ.get("KNOPOOL", "1") == "1"
KNOABSR = os.environ.get("KNOABSR", "0") == "1"
KSTTACC = os.environ.get("KSTTACC", "1") == "1"

N, D, T, E, NC = 384, 2, 128, 256, 8
NROW = N // NC          # 48 rows per core
DMAX = 191
SP_PER = 192 // NC      # 24 strip pairs per core
EV_PER = E // NC        # 32 events per core
EXTW = NROW + DMAX + 1  # 240 ext stream window
G = int(os.environ.get("KG", "8"))  # rows per chunk
NCHUNK = NROW // G
F = G * DMAX
FLAST = F + SP_PER
PHASE_GROUP = int(os.environ.get("KPG", "3"))
A2_FLOOR = 1e-12

_CACHE = {}
_OPS = {}


def _register_custom_ops():
    """Register fused DVE ops (idempotent). PAIR_SQDIST computes
    max((x-c0)^2 + (y-c1)^2, imm2) in one pass; DIFF_PROD computes
    (x-c0)*(y-c1)."""
    global _OPS
    if _OPS:
        return _OPS
    import concourse.dve_ops as dve_ops
    from concourse.dve_spec import Spec, Src0, Src1, C0, C1, C2, sq, maxx, lower
    from concourse.dve_uop import DveOpSpec

    def make(name, spec):
        for op in dve_ops.OPS:
            if op.name == name:
                return op
        shas = {}
        for ver in ("v3", "v4"):
            try:
                s = DveOpSpec(name=name, opcode=1, uops=lower(spec, ver=ver),
                              rd1_en=True)
                shas[ver] = s.sha(ver)
            except Exception:
                pass
        op = dve_ops.DveOp(name, spec, subdim=False, uops_sha=shas)
        dve_ops.OPS.append(op)
        dve_ops.CUSTOM_DVE_SPECS[op.name] = op.spec
        dve_ops._SUB_OPCODE_FOR_NAME[op.name] = (
            dve_ops._CUSTOM_DVE_ROW_BASE + len(dve_ops.OPS) - 1)
        return op

    def _sqd_ref(in0, in1, c0, c1, c2):
        a = (in0.astype(np.float32) - c0) ** 2 + (in1.astype(np.float32) - c1) ** 2
        return np.maximum(a, c2)

    def _dp_ref(in0, in1, c0, c1, c2):
        return (in0.astype(np.float32) - c0) * (in1.astype(np.float32) - c1)

    _OPS["SQD"] = make("PAIR_SQDIST",
                       Spec(body=maxx(sq(Src0 - C0) + sq(Src1 - C1), C2),
                            reference=_sqd_ref))
    _OPS["DP"] = make("DIFF_PROD",
                      Spec(body=(Src0 - C0) * (Src1 - C1), reference=_dp_ref))
    return _OPS


def _build_program(dt):
    from contextlib import ExitStack
    import concourse.bacc as bacc
    import concourse.tile as tile
    import concourse.mybir as mybir

    ops = _register_custom_ops()
    SQD, DP = ops["SQD"], ops["DP"]

    f32 = mybir.dt.float32
    bf16 = mybir.dt.bfloat16
    AF = mybir.ActivationFunctionType
    OP = mybir.AluOpType
    AX = mybir.AxisListType

    nc = bacc.Bacc("TRN2", target_bir_lowering=False, debug=False, num_devices=NC)

    def din(name, shape, dty=f32):
        return nc.dram_tensor(name, shape, dty, kind="ExternalInput").ap()

    # fp32 streams [T, EXTW]: v, z (step-start positions), per component
    # (fp32 inputs only: bf16 DRAM inputs crash the PJRT path; custom DVE
    # ops run at 1x regardless of dtype so fp32 stream reads are free)
    vx_d = din("vx", [T, EXTW])
    vy_d = din("vy", [T, EXTW])
    zx_d = din("zx", [T, EXTW])
    zy_d = din("zy", [T, EXTW])
    # strip tensors [T, 8*SP_PER]: vxA|vxB|vyA|vyB|zxA|zxB|zyA|zyB
    strip_d = din("strip", [T, 8 * SP_PER])
    # fp32 left-node columns (custom-DVE scalar slots require fp32)
    vxc_d = din("vxc", [T, NROW])
    vyc_d = din("vyc", [T, NROW])
    zxc_d = din("zxc", [T, NROW])
    zyc_d = din("zyc", [T, NROW])
    nzxc_d = din("nzxc", [T, NROW])
    nzyc_d = din("nzyc", [T, NROW])
    # event machinery (fp32, as in baseline)
    vnatx_d = din("vnatx", [N, T])
    vnaty_d = din("vnaty", [N, T])
    qmat_d = din("qmat", [N, EV_PER])
    wmat_d = din("wmat", [T, EV_PER])
    bhot_d = din("bhot", [T, EV_PER])
    dz0x_d = din("dz0x", [1, EV_PER])
    dz0y_d = din("dz0y", [1, EV_PER])
    betac_d = din("betac", [T, 1])
    betaln_d = din("betaln", [T, 1])
    out_d = nc.dram_tensor("out", [1, 2], f32, kind="ExternalOutput").ap()

    with ExitStack() as ctx:
        tc = ctx.enter_context(tile.TileContext(nc))
        sg = ctx.enter_context(tc.tile_pool(name="singles", bufs=1))
        wk = ctx.enter_context(tc.tile_pool(name="work", bufs=PHASE_GROUP + 1))
        ps = ctx.enter_context(tc.tile_pool(name="psum", bufs=1, space="PSUM"))

        def load(dram, shape, tag, dty=f32):
            t = sg.tile(shape, dty, name=tag, tag=tag)
            nc.sync.dma_start(out=t[:], in_=dram[:])
            return t

        vx = load(vx_d, [T, EXTW], "vx")
        vy = load(vy_d, [T, EXTW], "vy")
        zx = load(zx_d, [T, EXTW], "zx")
        zy = load(zy_d, [T, EXTW], "zy")
        strip = load(strip_d, [T, 8 * SP_PER], "strip")
        vxc = load(vxc_d, [T, NROW], "vxc")
        vyc = load(vyc_d, [T, NROW], "vyc")
        zxc = load(zxc_d, [T, NROW], "zxc")
        zyc = load(zyc_d, [T, NROW], "zyc")
        nzxc = load(nzxc_d, [T, NROW], "nzxc")
        nzyc = load(nzyc_d, [T, NROW], "nzyc")
        vnx = [sg.tile([128, T], f32, name=f"vnx{r}", tag=f"vnx{r}") for r in range(3)]
        vny = [sg.tile([128, T], f32, name=f"vny{r}", tag=f"vny{r}") for r in range(3)]
        qm = [sg.tile([128, EV_PER], f32, name=f"qm{r}", tag=f"qm{r}") for r in range(3)]
        for r in range(3):
            nc.sync.dma_start(out=vnx[r][:], in_=vnatx_d[128 * r:128 * (r + 1), :])
            nc.sync.dma_start(out=vny[r][:], in_=vnaty_d[128 * r:128 * (r + 1), :])
            nc.sync.dma_start(out=qm[r][:], in_=qmat_d[128 * r:128 * (r + 1), :])
        wmat = load(wmat_d, [T, EV_PER], "wmat")
        bhot = load(bhot_d, [T, EV_PER], "bhot")
        dz0x = load(dz0x_d, [1, EV_PER], "dz0x")
        dz0y = load(dz0y_d, [1, EV_PER], "dz0y")
        betac = load(betac_d, [T, 1], "betac")
        betaln = load(betaln_d, [T, 1], "betaln")

        ones = sg.tile([T, 1], f32)
        nc.vector.memset(ones[:], 1.0)
        cm1 = sg.tile([T, 1], f32)
        nc.vector.memset(cm1[:], -1.0)
        cdt = sg.tile([T, 1], f32)
        nc.vector.memset(cdt[:], float(dt))
        czero = sg.tile([T, 1], f32)
        nc.vector.memset(czero[:], 0.0)
        ecol = sg.tile([T, 1], f32)
        nc.vector.memset(ecol[:], float(A2_FLOOR * dt * dt))

        # ---- main pipeline ----
        tout_shared = sg.tile([T, FLAST], bf16, name="tout_shared", tag="tout_shared")
        if KNOCUSTOM:
            diffs_shared = [sg.tile([T, FLAST], f32, name=f"dif{k}", tag=f"dif{k}")
                            for k in range(4)]
        accs = []

        def chunk_tiles(ci):
            fw = FLAST if ci == 0 else F
            t = {}
            for nm in ("a2", "r2", "bx", "by", "b", "sa", "usq", "w", "exiv", "K"):
                t[nm] = wk.tile([T, FLAST], bf16, name=f"{nm}{ci % (PHASE_GROUP+1)}",
                                tag=nm)
            t["inva"] = wk.tile([T, FLAST], bf16, name=f"inva{ci}", tag="inva")
            t["argp"] = wk.tile([T, 2 * FLAST], bf16, name=f"argp{ci}", tag="argp")
            t["EP"] = wk.tile([T, 2 * FLAST], bf16, name=f"EP{ci}", tag="EP")
            t["tout"] = tout_shared
            t["fw"] = fw
            return t

        def emit_sqd_phase(ci, t):
            fw = t["fw"]
            if KNOCUSTOM:
                dvx, dvy, dzx, dzy = diffs_shared
                for g in range(G):
                    r = ci * G + g
                    cs = slice(r + 1, r + 192)
                    o = slice(191 * g, 191 * (g + 1))
                    rc = slice(r, r + 1)
                    nc.vector.tensor_scalar(out=dvx[:, o], in0=vx[:, cs], scalar1=vxc[:, rc], scalar2=None, op0=OP.subtract)
                    nc.vector.tensor_scalar(out=dvy[:, o], in0=vy[:, cs], scalar1=vyc[:, rc], scalar2=None, op0=OP.subtract)
                    nc.vector.tensor_scalar(out=dzx[:, o], in0=zx[:, cs], scalar1=zxc[:, rc], scalar2=None, op0=OP.subtract)
                    nc.vector.tensor_scalar(out=dzy[:, o], in0=zy[:, cs], scalar1=zyc[:, rc], scalar2=None, op0=OP.subtract)
                nc.vector.tensor_mul(t["a2"][:, :F], dvx[:, :F], dvx[:, :F])
                nc.vector.tensor_mul(t["usq"][:, :F], dvy[:, :F], dvy[:, :F])
                nc.vector.tensor_add(t["a2"][:, :F], t["a2"][:, :F], t["usq"][:, :F])
                nc.vector.tensor_scalar_max(out=t["a2"][:, :F], in0=t["a2"][:, :F], scalar1=A2_FLOOR)
                nc.vector.tensor_mul(t["r2"][:, :F], dzx[:, :F], dzx[:, :F])
                nc.vector.tensor_mul(t["usq"][:, :F], dzy[:, :F], dzy[:, :F])
                nc.vector.tensor_add(t["r2"][:, :F], t["r2"][:, :F], t["usq"][:, :F])
                nc.vector.tensor_mul(t["bx"][:, :F], dzx[:, :F], dvx[:, :F])
                nc.vector.tensor_mul(t["by"][:, :F], dzy[:, :F], dvy[:, :F])
                if ci == NCHUNK - 1:
                    sA = lambda k: strip[:, (2 * k) * SP_PER:(2 * k + 1) * SP_PER]
                    sB = lambda k: strip[:, (2 * k + 1) * SP_PER:(2 * k + 2) * SP_PER]
                    o = slice(F, FLAST)
                    nc.vector.tensor_sub(dvx[:, o], sA(0), sB(0))
                    nc.vector.tensor_sub(dvy[:, o], sA(1), sB(1))
                    nc.vector.tensor_sub(dzx[:, o], sA(2), sB(2))
                    nc.vector.tensor_sub(dzy[:, o], sA(3), sB(3))
                    nc.vector.tensor_mul(t["a2"][:, o], dvx[:, o], dvx[:, o])
                    nc.vector.tensor_mul(t["usq"][:, o], dvy[:, o], dvy[:, o])
                    nc.vector.tensor_add(t["a2"][:, o], t["a2"][:, o], t["usq"][:, o])
                    nc.vector.tensor_scalar_max(out=t["a2"][:, o], in0=t["a2"][:, o], scalar1=A2_FLOOR)
                    nc.vector.tensor_mul(t["r2"][:, o], dzx[:, o], dzx[:, o])
                    nc.vector.tensor_mul(t["usq"][:, o], dzy[:, o], dzy[:, o])
                    nc.vector.tensor_add(t["r2"][:, o], t["r2"][:, o], t["usq"][:, o])
                    nc.vector.tensor_mul(t["bx"][:, o], dzx[:, o], dvx[:, o])
                    nc.vector.tensor_mul(t["by"][:, o], dzy[:, o], dvy[:, o])
                return
            for g in range(G):
                r = ci * G + g
                cs = slice(r + 1, r + 192)
                o = slice(191 * g, 191 * (g + 1))
                rc = slice(r, r + 1)
                nc.vector._custom_dve(SQD, out=t["a2"][:, o], in0=vx[:, cs],
                                      in1=vy[:, cs], s0=vxc[:, rc], s1=vyc[:, rc],
                                      imm2=A2_FLOOR)
                nc.scalar.activation(t["usq"][:, o], zx[:, cs], AF.Square,
                                     bias=nzxc[:, rc])
                nc.scalar.activation(t["w"][:, o], zy[:, cs], AF.Square,
                                     bias=nzyc[:, rc])
                nc.vector._custom_dve(DP, out=t["bx"][:, o], in0=zx[:, cs],
                                      in1=vx[:, cs], s0=zxc[:, rc], s1=vxc[:, rc],
                                      imm2=0.0)
                nc.vector._custom_dve(DP, out=t["by"][:, o], in0=zy[:, cs],
                                      in1=vy[:, cs], s0=zyc[:, rc], s1=vyc[:, rc],
                                      imm2=0.0)
            if ci == 0:
                # strip pairs: columns F..F+24; operands packed in `strip`
                sA = lambda k: strip[:, (2 * k) * SP_PER:(2 * k + 1) * SP_PER]
                sB = lambda k: strip[:, (2 * k + 1) * SP_PER:(2 * k + 2) * SP_PER]
                o = slice(F, FLAST)
                dvxs = sg.tile([T, SP_PER], f32)
                dvys = sg.tile([T, SP_PER], f32)
                dzxs = sg.tile([T, SP_PER], f32)
                dzys = sg.tile([T, SP_PER], f32)
                nc.vector.tensor_sub(dvxs[:], sA(0), sB(0))
                nc.vector.tensor_sub(dvys[:], sA(1), sB(1))
                nc.vector.tensor_sub(dzxs[:], sA(2), sB(2))
                nc.vector.tensor_sub(dzys[:], sA(3), sB(3))
                nc.vector._custom_dve(SQD, out=t["a2"][:, o], in0=dvxs[:],
                                      in1=dvys[:], s0=0.0, s1=0.0, imm2=A2_FLOOR)
                nc.vector._custom_dve(SQD, out=t["r2"][:, o], in0=dzxs[:],
                                      in1=dzys[:], s0=0.0, s1=0.0, imm2=0.0)
                nc.vector._custom_dve(DP, out=t["bx"][:, o], in0=dzxs[:],
                                      in1=dvxs[:], s0=0.0, s1=0.0, imm2=0.0)
                nc.vector._custom_dve(DP, out=t["by"][:, o], in0=dzys[:],
                                      in1=dvys[:], s0=0.0, s1=0.0, imm2=0.0)

        def emit_b(ci, t):
            fw = t["fw"]
            if ci == 0:
                r2sl = slice(0, F)
            else:
                r2sl = slice(0, fw)
            nc.vector.tensor_add(t["r2"][:, r2sl], t["usq"][:, r2sl], t["w"][:, r2sl])
            if KNOPOOL:
                nc.vector.tensor_add(t["b"][:, :fw], t["bx"][:, :fw], t["by"][:, :fw])
            else:
                nc.gpsimd.tensor_add(t["b"][:, :fw], t["bx"][:, :fw], t["by"][:, :fw])

        def emit_absr(ci, t):
            fw = t["fw"]
            nc.scalar.activation(t["inva"][:, :fw], t["a2"][:, :fw],
                                 AF.Abs_reciprocal_sqrt,
                                 scale=float(dt * dt), bias=ecol[:])

        def emit_args(ci, t):
            fw = t["fw"]
            # u' = u/dt = b*invad; px = arg1/dt = (a2*dt + b)*invad
            nc.vector.tensor_mul(t["argp"][:, fw:2 * fw], t["b"][:, :fw],
                                 t["inva"][:, :fw])
            nc.scalar.mul(t["sa"][:, :fw], t["a2"][:, :fw], cdt[:])
            nc.vector.tensor_add(t["sa"][:, :fw], t["sa"][:, :fw], t["b"][:, :fw])
            nc.vector.tensor_mul(t["argp"][:, 0:fw], t["sa"][:, :fw],
                                 t["inva"][:, :fw])

        def emit_erf(ci, t):
            fw = t["fw"]
            nc.scalar.activation(t["EP"][:, 0:2 * fw], t["argp"][:, 0:2 * fw], AF.Erf,
                                 scale=float(dt), bias=czero[:])

        def emit_usq(ci, t):
            fw = t["fw"]
            # usq = (dt*u')^2 on ACT (Square lives in every table set)
            nc.scalar.activation(t["usq"][:, :fw], t["argp"][:, fw:2 * fw], AF.Square,
                                 scale=float(dt), bias=czero[:])

        def emit_w(ci, t):
            fw = t["fw"]
            if KNOPOOL:
                nc.vector.tensor_sub(t["w"][:, :fw], t["r2"][:, :fw], t["usq"][:, :fw])
            else:
                nc.gpsimd.tensor_sub(t["w"][:, :fw], t["r2"][:, :fw], t["usq"][:, :fw])

        def emit_exp(ci, t):
            fw = t["fw"]
            nc.scalar.activation(t["exiv"][:, :fw], t["w"][:, :fw], AF.Exp,
                                 bias=betaln[:], scale=-1.0)

        def emit_tail(ci, t):
            fw = t["fw"]
            nc.vector.tensor_mul(t["K"][:, :fw], t["exiv"][:, :fw], t["inva"][:, :fw])
            a1 = sg.tile([T, 1], f32, name=f"acc1_{ci}", tag=f"acc1_{ci}")
            nc.vector.tensor_sub(t["usq"][:, :fw], t["EP"][:, 0:fw],
                                 t["EP"][:, fw:2 * fw])
            nc.vector.scalar_tensor_tensor(out=t["tout"][:, :fw],
                                           in0=t["usq"][:, :fw], scalar=ones[:],
                                           in1=t["K"][:, :fw],
                                           op0=OP.mult, op1=OP.mult,
                                           accum_out=a1[:])
            accs.append(a1)

        groups = [list(range(s, min(s + PHASE_GROUP, NCHUNK)))
                  for s in range(0, NCHUNK, PHASE_GROUP)]
        tiles = {}
        for grp in groups:
            for ci in grp:
                tiles[ci] = chunk_tiles(ci)
                emit_sqd_phase(ci, tiles[ci])
                emit_b(ci, tiles[ci])
            for ci in grp:
                emit_absr(ci, tiles[ci])
            for ci in grp:
                emit_args(ci, tiles[ci])
            for ci in grp:
                emit_erf(ci, tiles[ci])
            for ci in grp:
                emit_usq(ci, tiles[ci])
            for ci in grp:
                emit_w(ci, tiles[ci])
            for ci in grp:
                emit_exp(ci, tiles[ci])
            for ci in grp:
                emit_tail(ci, tiles[ci])

        # ---- event term (one-hot gather matmuls on PE) ----
        gdx = ps.tile([T, EV_PER], f32)
        gdy = ps.tile([T, EV_PER], f32)
        for r in range(3):
            nc.tensor.matmul(gdx[:], vnx[r][:], qm[r][:],
                             start=(r == 0), stop=(r == 2))
        for r in range(3):
            nc.tensor.matmul(gdy[:], vny[r][:], qm[r][:],
                             start=(r == 0), stop=(r == 2))
        hx = sg.tile([T, EV_PER], f32)
        hy = sg.tile([T, EV_PER], f32)
        nc.vector.tensor_mul(hx[:], gdx[:], wmat[:])
        nc.vector.tensor_mul(hy[:], gdy[:], wmat[:])
        shx = ps.tile([1, EV_PER], f32)
        shy = ps.tile([1, EV_PER], f32)
        brow = ps.tile([1, EV_PER], f32)
        nc.tensor.matmul(shx[:], ones[:], hx[:])
        nc.tensor.matmul(shy[:], ones[:], hy[:])
        nc.tensor.matmul(brow[:], betac[:], bhot[:])
        evx = sg.tile([1, EV_PER], f32)
        evy = sg.tile([1, EV_PER], f32)
        nc.vector.tensor_add(evx[:], shx[:], dz0x[:])
        nc.vector.tensor_add(evy[:], shy[:], dz0y[:])
        nc.vector.tensor_mul(evx[:], evx[:], evx[:])
        nc.vector.tensor_mul(evy[:], evy[:], evy[:])
        nc.vector.tensor_add(evx[:], evx[:], evy[:])
        evel = sg.tile([1, EV_PER], f32)
        nc.vector.tensor_sub(evel[:], brow[:], evx[:])
        ev_s = sg.tile([1, 1], f32)
        nc.vector.reduce_sum(out=ev_s[:], in_=evel[:], axis=AX.X)


        # ---- reduce partials: sum 12 [T,1] cols, then partition-reduce on PE
        while len(accs) > 1:
            nxt = []
            for i in range(0, len(accs) - 1, 2):
                dst = sg.tile([T, 1], f32, name=f"red{len(nxt)}_{len(accs)}",
                              tag=f"red{len(nxt)}_{len(accs)}")
                nc.vector.tensor_add(dst[:], accs[i][:], accs[i + 1][:])
                nxt.append(dst)
            if len(accs) % 2:
                nxt.append(accs[-1])
            accs = nxt
        s_ps = ps.tile([1, 1], f32)
        nc.tensor.matmul(s_ps[:], accs[0][:], ones[:])
        out_sb = sg.tile([1, 2], f32)
        nc.vector.tensor_copy(out_sb[:, 0:1], ev_s[:])
        nc.vector.tensor_copy(out_sb[:, 1:2], s_ps[:])
        nc.sync.dma_start(out=out_d[:], in_=out_sb[:])

    nc.finalize()
    return nc


def _host_prep(data, t0, tn, z0, v0, beta):
    dt = float(tn - t0) / T
    v0x = np.ascontiguousarray(v0[:, 0, :]).astype(np.float64)  # [N, T]
    v0y = np.ascontiguousarray(v0[:, 1, :]).astype(np.float64)
    # step-start positions Zs[n, t] = z0 + sum_{k<t} v*dt
    zsx = z0[:, 0:1].astype(np.float64) + np.concatenate(
        [np.zeros((N, 1)), np.cumsum(v0x * dt, axis=1)[:, :-1]], axis=1)
    zsy = z0[:, 1:2].astype(np.float64) + np.concatenate(
        [np.zeros((N, 1)), np.cumsum(v0y * dt, axis=1)[:, :-1]], axis=1)

    times = data[:, 2]
    idx_f = np.floor(times / dt)
    idx = np.where(idx_f < T, idx_f, idx_f - 1.0).astype(np.int32)
    rem = (times - idx_f * dt).astype(np.float32)
    i_idx = np.floor(data[:, 0]).astype(np.int32)
    j_idx = np.floor(data[:, 1]).astype(np.int32)
    z0x, z0y = z0[:, 0], z0[:, 1]

    betaln = (beta + np.log(np.sqrt(np.pi) / 2.0) + np.log(dt)).astype(np.float32)[:, None]

    in_maps = []
    for c in range(NC):
        w = (np.arange(EXTW) + NROW * c) % N
        rows = w[:NROW]
        m = {
            "vx": v0x[w, :].T.astype(np.float32), "vy": v0y[w, :].T.astype(np.float32),
            "zx": zsx[w, :].T.astype(np.float32), "zy": zsy[w, :].T.astype(np.float32),
            "vxc": v0x[rows, :].T.astype(np.float32),
            "vyc": v0y[rows, :].T.astype(np.float32),
            "zxc": zsx[rows, :].T.astype(np.float32),
            "zyc": zsy[rows, :].T.astype(np.float32),
            "nzxc": -zsx[rows, :].T.astype(np.float32),
            "nzyc": -zsy[rows, :].T.astype(np.float32),
            "vnatx": v0x.astype(np.float32), "vnaty": v0y.astype(np.float32),
            "betac": np.ascontiguousarray(beta[:, None], np.float32),
            "betaln": betaln,
        }
        sA = np.arange(SP_PER * c, SP_PER * (c + 1))
        sB = sA + 192
        m["strip"] = np.concatenate(
            [v0x[sA].T, v0x[sB].T, v0y[sA].T, v0y[sB].T,
             zsx[sA].T, zsx[sB].T, zsy[sA].T, zsy[sB].T], axis=1).astype(np.float32)
        es = slice(EV_PER * c, EV_PER * (c + 1))
        ii, jj, dd, rr = i_idx[es], j_idx[es], idx[es], rem[es]
        Q = np.zeros((N, EV_PER), np.float32)
        W = np.zeros((T, EV_PER), np.float32)
        B = np.zeros((T, EV_PER), np.float32)
        for e in range(EV_PER):
            Q[ii[e], e] += 1.0
            Q[jj[e], e] -= 1.0
            W[:dd[e], e] = dt
            W[dd[e], e] += rr[e]
            B[dd[e], e] = 1.0
        m["qmat"], m["wmat"], m["bhot"] = Q, W, B
        m["dz0x"] = (z0x[ii] - z0x[jj])[None, :].astype(np.float32)
        m["dz0y"] = (z0y[ii] - z0y[jj])[None, :].astype(np.float32)
        mm = {}
        for k, v in m.items():
            mm[k] = np.ascontiguousarray(v)
        in_maps.append(mm)
    return dt, in_maps


def _run(inputs, trace=False):
    from concourse.bass_utils import run_bass_kernel_spmd
    data = np.asarray(inputs["data"], np.float32)
    t0 = float(np.asarray(inputs["t0"]))
    tn = float(np.asarray(inputs["tn"]))
    z0 = np.asarray(inputs["z0"], np.float32)
    v0 = np.asarray(inputs["v0"], np.float32)
    beta = np.asarray(inputs["beta"], np.float32)

    dt, in_maps = _host_prep(data, t0, tn, z0, v0, beta)
    if dt not in _CACHE:
        _CACHE[dt] = _build_program(dt)
    nc = _CACHE[dt]
    res = run_bass_kernel_spmd(nc, in_maps, core_ids=list(range(NC)), trace=trace)
    ev = sum(float(res.results[c]["out"][0, 0]) for c in range(NC))
    S = sum(float(res.results[c]["out"][0, 1]) for c in range(NC))
    return np.array(np.float32(ev - S)), res


def kernel(**inputs):
    out, _ = _run(inputs, trace=False)
    return out
